# revision 24
# baseline (speedup 1.0000x reference)
"""Multi-head attention (B=4, S=2048, D=1024, H=16, d_k=64) on 8 TRN2 NeuronCores.

Sharding: batch x head-group. Core c handles batch b = c//2 and heads
[8*(c%2), 8*(c%2)+8). Each core computes Q/K/V projections for its 512
output features (column-parallel), attention for its 8 heads, and a
row-parallel partial of the W_o output projection. The host sums the two
bf16 partials per batch (the row-parallel unshard) — no collectives.

Schedule: the attention exp stream on the ACT engine is the critical
path (256 x [128,1024] exp tiles ~ 272us busy). Everything else is
interleaved into the attention (qc, m) kb-loop so the PE stays just
ahead of ACT:
- lead-in: dummy-matmul HAM warmup bridges the input DMA window; DMAs
  are emitted m0-first / first-512-tokens-first so Q(m0,qc0)+K(m0) start
  after ~2.5MB; K(m1) also runs in the DMA shadow.
- per kb slot: scores row-pair (tile_position 2-way) -> filler matmul ->
  attnV pair lagging LAG=3 exps behind, so attnV never stalls on exp and
  weight loads hide under the filler's stream.
- the last LAG attnV matmuls + av evacuation of window W run in the
  first slots of window W+1 (ring) — no drain bubble at boundaries.
- V tiles are produced inside the (qc0, m0) window just-in-time for the
  attnV consumption order; K(m+2) projections self-feed inside window
  (qc0, m+1); Q chunks for the next qc self-feed one window ahead as a
  single compact thunk (short PSUM-slot lifetime); Wo matmuls spread
  over even slots one qc behind.
- softmax denominators ride the attn@V matmul as a 65th ones-column of
  V; reciprocal_approx_fast + deferred normalization via a tiny
  mask-matmul broadcast; output staged and DMA'd as bf16.
"""

import os

import numpy as np
import ml_dtypes

import concourse.bacc as bacc
import concourse.mybir as mybir
import concourse.tile as tile
from concourse.bass_utils import run_bass_kernel_spmd

BF16 = mybir.dt.bfloat16
F32 = mybir.dt.float32
EXP = mybir.ActivationFunctionType.Exp

B, S, D = 4, 2048, 1024
H, DK = 16, 64
HPC = 8           # heads per core
FPC = HPC * DK    # 512 features per core
NP = 4            # head pairs per core
NB = 8            # din blocks of 128
NKB = 16          # key blocks of 128
NQC = 4           # q chunks of 512
QC = 512
NTT = 16          # token tiles of 128
LAG = 3           # attnV trails exp by LAG kb steps

_nc_cache = None
last_results = None


def build():
    nc = bacc.Bacc("TRN2", target_bir_lowering=False, debug=False, num_devices=8)

    xq = nc.dram_tensor("xq", [D, S], BF16, kind="ExternalInput").ap()
    xk = nc.dram_tensor("xk", [D, S], BF16, kind="ExternalInput").ap()
    xv = nc.dram_tensor("xv", [D, S], BF16, kind="ExternalInput").ap()
    wq = nc.dram_tensor("wq", [D, FPC], BF16, kind="ExternalInput").ap()
    wk = nc.dram_tensor("wk", [D, FPC], BF16, kind="ExternalInput").ap()
    wv = nc.dram_tensor("wv", [D, FPC], BF16, kind="ExternalInput").ap()
    wo = nc.dram_tensor("wo", [FPC, D], BF16, kind="ExternalInput").ap()
    mask = nc.dram_tensor("mask", [2, 128], BF16, kind="ExternalInput").ap()
    out = nc.dram_tensor("out", [S, D], BF16, kind="ExternalOutput").ap()

    with tile.TileContext(nc) as tc:
        with (
            tc.tile_pool(name="wp", bufs=1) as wp,
            tc.tile_pool(name="qkv", bufs=1) as qkv,
            tc.tile_pool(name="ptp", bufs=5) as ptp,
            tc.tile_pool(name="otp", bufs=2) as otp,
            tc.tile_pool(name="smalls", bufs=2) as smalls,
            tc.tile_pool(name="outp", bufs=3) as outp,
            tc.tile_pool(name="xqp", bufs=2) as xqp,
            tc.tile_pool(name="xp", bufs=1) as xp,
            tc.tile_pool(name="sp", bufs=2, space="PSUM") as sp,
            tc.tile_pool(name="avp", bufs=2, space="PSUM") as avp,
            tc.tile_pool(name="miscp", bufs=2, space="PSUM") as miscp,
        ):
            wq_sb = wp.tile([128, NB, NP, 128], BF16, tag="wq")
            wk_sb = wp.tile([128, NB, NP, 128], BF16, tag="wk")
            wv_sb = wp.tile([128, NB, FPC], BF16, tag="wv")
            wo_sb = wp.tile([128, NP, D], BF16, tag="wo")
            m_sb = wp.tile([2, 128], BF16, tag="mask")
            warm = wp.tile([64, 64], BF16, tag="warm")

            qt_sb = qkv.tile([128, NP, S], BF16, tag="qt")
            kt_sb = qkv.tile([128, NP, S], BF16, tag="kt")
            v_sb = qkv.tile([128, NKB, HPC, 65], BF16, tag="v")
            nc.vector.memset(warm[:], 0.0)
            nc.vector.memset(v_sb[:, :, :, 64], 1.0)

            xk_sb = xp.tile([128, NB, S], BF16, tag="xk", name="xk_sb")
            xv_sb = xp.tile([128, NB, S], BF16, tag="xv", name="xv_sb")

            # xq lives as 512-token quarters (one per q chunk); quarters 2/3
            # are allocated+DMA'd mid-kernel reusing quarters 0/1's slots.
            xqq = {}

            def alloc_xq_quarter(c):
                t = xqp.tile([128, NB, QC], BF16, tag="xqh", name="xqh")
                xqq[c] = t
                for b in range(NB):
                    nc.sync.dma_start(
                        t[:, b], xq[b * 128:(b + 1) * 128,
                                    c * QC:(c + 1) * QC])

            # ---- DMA emission: priority order for the lead-in ----
            # m0-first weight slices and token-quarter x slices so the first
            # scores/exp only wait on ~2.5MB instead of the full 16MB.
            def bsl_(b):
                return slice(b * 128, (b + 1) * 128)

            nc.sync.dma_start(m_sb[:], mask)
            for b in range(NB):
                nc.sync.dma_start(wq_sb[:, b, 0], wq[bsl_(b), 0:128])
            alloc_xq_quarter(0)
            for b in range(NB):
                nc.sync.dma_start(wk_sb[:, b, 0], wk[bsl_(b), 0:128])
            for b in range(NB):
                nc.sync.dma_start(xk_sb[:, b, 0:QC], xk[bsl_(b), 0:QC])
            for b in range(NB):
                nc.sync.dma_start(xk_sb[:, b, QC:S], xk[bsl_(b), QC:S])
            for b in range(NB):
                nc.sync.dma_start(
                    wk_sb[:, b, 1:4],
                    wk[bsl_(b), 128:512].rearrange("p (m c) -> p m c", c=128))
            for b in range(NB):
                nc.sync.dma_start(wv_sb[:, b], wv[bsl_(b), :])
            for b in range(NB):
                nc.sync.dma_start(xv_sb[:, b, 0:QC], xv[bsl_(b), 0:QC])
            for c in range(1, 4):
                for b in range(NB):
                    nc.sync.dma_start(xv_sb[:, b, c * QC:(c + 1) * QC],
                                      xv[bsl_(b), c * QC:(c + 1) * QC])
            for b in range(NB):
                nc.sync.dma_start(
                    wq_sb[:, b, 1:4],
                    wq[bsl_(b), 128:512].rearrange("p (m c) -> p m c", c=128))
            alloc_xq_quarter(1)
            for fb in range(NP):
                nc.sync.dma_start(wo_sb[:, fb], wo[fb * 128:(fb + 1) * 128, :])

            # ---- filler thunk builders ----
            def q_chunk_thunk(m, c):
                def th(m=m, c=c):
                    ps = miscp.tile([128, QC], F32, tag="misc", name="pq")
                    for b in range(NB):
                        nc.tensor.matmul(
                            ps[:], wq_sb[:, b, m], xqq[c][:, b, :],
                            start=(b == 0), stop=(b == NB - 1))
                    nc.vector.tensor_copy(
                        qt_sb[:, m, c * QC:(c + 1) * QC], ps[:])
                return th

            def k_chunk_mms(m, c):
                st = {}
                ths = []
                for b in range(NB):
                    def mm(b=b, m=m, c=c):
                        if b == 0:
                            st["ps"] = miscp.tile([128, QC], F32, tag="misc",
                                                  name="pk")
                        nc.tensor.matmul(
                            st["ps"][:], wk_sb[:, b, m],
                            xk_sb[:, b, c * QC:(c + 1) * QC],
                            start=(b == 0), stop=(b == NB - 1))
                        if b == NB - 1:
                            nc.vector.tensor_copy(
                                kt_sb[:, m, c * QC:(c + 1) * QC], st["ps"][:])
                    ths.append(mm)
                return ths

            def v_tt_thunk(tt):
                def th(tt=tt):
                    ps = miscp.tile([128, FPC], F32, tag="misc", name="pv")
                    for b in range(NB):
                        nc.tensor.matmul(
                            ps[:], xv_sb[:, b, tt * 128:(tt + 1) * 128],
                            wv_sb[:, b],
                            start=(b == 0), stop=(b == NB - 1))
                    nc.vector.tensor_copy(
                        v_sb[:, tt, :, 0:64],
                        ps[:].rearrange("p (h c) -> p h c", c=64))
                return th

            ot_tiles = {}

            def wo_thunks(qc_w, tt):
                ot_w = ot_tiles[qc_w]
                st = {}
                ths = []
                for jc in range(2):
                    for fb in range(NP):
                        def mm(jc=jc, fb=fb, qc_w=qc_w, tt=tt):
                            if fb == 0:
                                st[jc] = miscp.tile([128, QC], F32, tag="misc",
                                                    name="wop")
                                if jc == 0:
                                    st["o"] = outp.tile([128, D], BF16,
                                                        tag="ostage",
                                                        name="ostage")
                            tsl = slice(tt * 128, (tt + 1) * 128)
                            nc.tensor.matmul(
                                st[jc][:], ot_w[fb][:, tsl],
                                wo_sb[:, fb, jc * 512:(jc + 1) * 512],
                                start=(fb == 0), stop=(fb == NP - 1))
                            if fb == NP - 1:
                                nc.vector.tensor_copy(
                                    st["o"][:, jc * 512:(jc + 1) * 512],
                                    st[jc][:])
                                if jc == 1:
                                    row = qc_w * QC + tt * 128
                                    nc.sync.dma_start(out[row:row + 128, :],
                                                      st["o"][:])
                        ths.append(mm)
                return ths

            def finish_pair(job):
                ot_t, av_sb_t, rec2_t = job
                scp = miscp.tile([128, QC], F32, tag="misc", name="scp")
                nc.tensor.matmul(scp[:], m_sb[:], rec2_t[:], start=True,
                                 stop=True)
                nc.vector.tensor_mul(ot_t[0:64, :], av_sb_t[0:64, 0:QC],
                                     scp[0:64, :])
                nc.vector.tensor_mul(ot_t[64:128, :],
                                     av_sb_t[0:64, QC:2 * QC], scp[64:128, :])

            def put(fill, start_slot, thunks, per_slot=2):
                for i, th in enumerate(thunks):
                    fill[min(start_slot + i // per_slot, NKB - 1)].append(th)

            def put_stride(fill, thunks, first, stride):
                for i, th in enumerate(thunks):
                    fill[min(first + stride * i, NKB - 1)].append(th)

            # ---- lead-in ----
            # ~160 dummy matmuls (~9us) keep the PE HAM-warm while the input
            # DMAs land, so the real projections start at 2.4GHz.
            NWARM = 160
            wps = miscp.tile([64, 64], F32, tag="misc", name="warmps")
            for i in range(NWARM):
                nc.tensor.matmul(wps[0:64, 0:64], warm[:], warm[:],
                                 start=(i == 0), stop=(i == NWARM - 1))
            q_chunk_thunk(0, 0)()
            for c in range(4):
                for th in k_chunk_mms(0, c):
                    th()
            for c in range(4):
                for th in k_chunk_mms(1, c):
                    th()
            for tt in range(4):
                v_tt_thunk(tt)()

            # ---- attention (qc, m) with interleaved fillers ----
            # The last LAG attnV matmuls + av evacuation of window W run in
            # the first slots of window W+1 (ring), so the PE never drains
            # waiting for W's last exp before starting W+1's scores.
            pending = None
            carry = []
            for qc in range(NQC):
                ot_tiles[qc] = {}
                qsl = slice(qc * QC, (qc + 1) * QC)
                for m in range(NP):
                    ot = otp.tile([128, QC], BF16, tag="ot%d" % m, name="ot")
                    ot_tiles[qc][m] = ot
                    fill = [[] for _ in range(NKB)]
                    if qc == 0:
                        if m == 0:
                            for i, tt in enumerate(range(4, 16)):
                                fill[i].append(v_tt_thunk(tt))
                            fill[12].append(q_chunk_thunk(1, 0))
                        else:
                            if m < NP - 1:
                                for c in range(4):
                                    put(fill, c * 3, k_chunk_mms(m + 1, c),
                                        per_slot=3)
                                fill[12].append(q_chunk_thunk(m + 1, 0))
                            else:
                                fill[12].append(q_chunk_thunk(0, 1))
                    else:
                        if m == 0 and qc >= 1 and qc + 1 < NQC:
                            fill[0].append(
                                lambda c=qc + 1: alloc_xq_quarter(c))
                        put_stride(fill, wo_thunks(qc - 1, m), 2, 2)
                        if m < NP - 1:
                            fill[9].append(q_chunk_thunk(m + 1, qc))
                        elif qc < NQC - 1:
                            fill[9].append(q_chunk_thunk(0, qc + 1))

                    avA = avp.tile([128, QC], F32, tag="av", name="avA")
                    avB = avp.tile([128, QC], F32, tag="av", name="avB")
                    pts = {}

                    def do_attnv(j, m=m, avA=avA, avB=avB, pts=pts):
                        nc.tensor.matmul(
                            avA[0:65, :], v_sb[:, j, 2 * m, 0:65],
                            pts[j][:, 0:512],
                            start=(j == 0), stop=(j == NKB - 1))
                        nc.tensor.matmul(
                            avB[0:65, :], v_sb[:, j, 2 * m + 1, 0:65],
                            pts[j][:, 512:1024],
                            start=(j == 0), stop=(j == NKB - 1))

                    def evac_pair(ot=ot, avA=avA, avB=avB):
                        nonlocal pending
                        av_sb = smalls.tile([128, 1024], F32, tag="av_sb",
                                            name="av_sb")
                        nc.vector.tensor_copy(av_sb[0:65, 0:QC], avA[0:65, :])
                        nc.vector.tensor_copy(av_sb[0:65, QC:2 * QC],
                                              avB[0:65, :])
                        den2 = smalls.tile([2, QC], F32, tag="den2",
                                           name="den2")
                        nc.sync.dma_start(den2[0:2, :], av_sb[64:65, 0:2 * QC])
                        recf = smalls.tile([2, QC], F32, tag="recf",
                                           name="recf")
                        nc.vector.reciprocal_approx_fast(recf[:], den2[:])
                        rec2 = smalls.tile([2, QC], BF16, tag="rec2",
                                           name="rec2")
                        nc.vector.tensor_copy(rec2[:], recf[:])
                        pending = (ot, av_sb, rec2)

                    for kb in range(NKB):
                        s = sp.tile([128, 1024], F32, tag="s", name="s")
                        ksl = slice(kb * 128, (kb + 1) * 128)
                        nc.tensor.matmul(s[:, 0:512], kt_sb[0:64, m, ksl],
                                         qt_sb[0:64, m, qsl],
                                         start=True, stop=True,
                                         tile_position=(0, 0))
                        nc.tensor.matmul(s[:, 512:1024], kt_sb[64:128, m, ksl],
                                         qt_sb[64:128, m, qsl],
                                         start=True, stop=True,
                                         tile_position=(64, 0))
                        pt = ptp.tile([128, 1024], BF16, tag="pt", name="pt")
                        nc.scalar.activation(pt[:], s[:], EXP, scale=0.125)
                        pts[kb] = pt
                        # order: scores -> fillers -> attnV. The scores
                        # row-pair streams 216ns but needs 2x107ns of weight
                        # loads; a filler matmul in between gives the load
                        # path stream-time to hide the attnV weight loads.
                        if kb < len(carry):
                            carry[kb]()
                        if kb == 8 and pending is not None:
                            finish_pair(pending)
                            pending = None
                        for f in fill[kb]:
                            f()
                        if kb >= LAG:
                            do_attnv(kb - LAG)

                    def mk_drain(j, last, do_attnv=do_attnv,
                                 evac_pair=evac_pair):
                        def d():
                            do_attnv(j)
                            if last:
                                evac_pair()
                        return d

                    carry = [mk_drain(j, j == NKB - 1)
                             for j in range(NKB - LAG, NKB)]

            # drain: last pair, its normalization, last q chunk's Wo.
            # tt0's fb0-2 matmuls only read already-normalized pairs, so they
            # run while the last pair's reciprocal chain completes.
            for th in carry:
                th()
            wps2 = miscp.tile([64, 64], F32, tag="misc", name="warmps2")
            for i in range(20):
                nc.tensor.matmul(wps2[0:64, 0:64], warm[:], warm[:],
                                 start=(i == 0), stop=(i == 19))
            tail0 = wo_thunks(NQC - 1, 0)
            for i in (0, 1, 2):
                tail0[i]()
            finish_pair(pending)
            for i in (3, 4, 5, 6, 7):
                tail0[i]()
            for tt in range(1, 4):
                for th in wo_thunks(NQC - 1, tt):
                    th()

    nc.compile()
    return nc


def _get_nc():
    global _nc_cache
    if _nc_cache is None:
        _nc_cache = build()
    return _nc_cache


def kernel(query, key, value, W_q, W_k, W_v, W_o):
    global last_results
    nc = _get_nc()
    bf = ml_dtypes.bfloat16

    mask = np.zeros((2, 128), bf)
    mask[0, 0:64] = 1.0
    mask[1, 64:128] = 1.0

    in_maps = []
    xt = {}
    for b in range(B):
        xt[b] = {
            "xq": np.ascontiguousarray(query[b].T).astype(bf),
            "xk": np.ascontiguousarray(key[b].T).astype(bf),
            "xv": np.ascontiguousarray(value[b].T).astype(bf),
        }
    wmaps = []
    for hg in range(2):
        r = slice(hg * FPC, (hg + 1) * FPC)
        wmaps.append({
            "wq": np.ascontiguousarray(W_q[r, :].T).astype(bf),
            "wk": np.ascontiguousarray(W_k[r, :].T).astype(bf),
            "wv": np.ascontiguousarray(W_v[r, :].T).astype(bf),
            "wo": np.ascontiguousarray(W_o[:, r].T).astype(bf),
        })
    for c in range(8):
        b, hg = c // 2, c % 2
        in_maps.append({**xt[b], **wmaps[hg], "mask": mask})

    res = run_bass_kernel_spmd(
        nc, in_maps, core_ids=list(range(8)),
        trace=bool(os.environ.get("BASS_KERNEL_TRACE")))
    last_results = res

    out = np.empty((B, S, D), np.float32)
    for b in range(B):
        out[b] = (res.results[2 * b]["out"].astype(np.float32)
                  + res.results[2 * b + 1]["out"].astype(np.float32))
    return out


# revision 25
# speedup vs baseline: 1.0160x; 1.0160x over previous
"""Multi-head attention (B=4, S=2048, D=1024, H=16, d_k=64) on 8 TRN2 NeuronCores.

Sharding: batch x head-group. Core c handles batch b = c//2 and heads
[8*(c%2), 8*(c%2)+8). Each core computes Q/K/V projections for its 512
output features (column-parallel), attention for its 8 heads, and a
row-parallel partial of the W_o output projection. The host sums the two
bf16 partials per batch (the row-parallel unshard) — no collectives.

Schedule: the attention exp stream on the ACT engine is the critical
path (256 x [128,1024] exp tiles ~ 272us busy). Everything else is
interleaved into the attention (qc, m) kb-loop so the PE stays just
ahead of ACT:
- lead-in: dummy-matmul HAM warmup bridges the input DMA window; DMAs
  are emitted m0-first / first-512-tokens-first so Q(m0,qc0)+K(m0) start
  after ~2.5MB; K(m1) also runs in the DMA shadow.
- per kb slot: scores row-pair (tile_position 2-way) -> filler matmul ->
  attnV pair lagging LAG=3 exps behind, so attnV never stalls on exp and
  weight loads hide under the filler's stream.
- the last LAG attnV matmuls + av evacuation of window W run in the
  first slots of window W+1 (ring) — no drain bubble at boundaries.
- V tiles are produced inside the (qc0, m0) window just-in-time for the
  attnV consumption order; K(m+2) projections self-feed inside window
  (qc0, m+1); Q chunks for the next qc self-feed one window ahead as a
  single compact thunk (short PSUM-slot lifetime); Wo matmuls spread
  over even slots one qc behind.
- softmax denominators ride the attn@V matmul as a 65th ones-column of
  V; reciprocal_approx_fast + deferred normalization via a tiny
  mask-matmul broadcast; output staged and DMA'd as bf16.
"""

import os

import numpy as np
import ml_dtypes

import concourse.bacc as bacc
import concourse.mybir as mybir
import concourse.tile as tile
from concourse.bass_utils import run_bass_kernel_spmd

BF16 = mybir.dt.bfloat16
F32 = mybir.dt.float32
EXP = mybir.ActivationFunctionType.Exp

B, S, D = 4, 2048, 1024
H, DK = 16, 64
HPC = 8           # heads per core
FPC = HPC * DK    # 512 features per core
NP = 4            # head pairs per core
NB = 8            # din blocks of 128
NKB = 16          # key blocks of 128
NQC = 4           # q chunks of 512
QC = 512
NTT = 16          # token tiles of 128
LAG = 3           # attnV trails exp by LAG kb steps

_nc_cache = None
last_results = None


def build():
    nc = bacc.Bacc("TRN2", target_bir_lowering=False, debug=False, num_devices=8)

    xq = nc.dram_tensor("xq", [D, S], BF16, kind="ExternalInput").ap()
    xk = nc.dram_tensor("xk", [D, S], BF16, kind="ExternalInput").ap()
    xv = nc.dram_tensor("xv", [D, S], BF16, kind="ExternalInput").ap()
    wq = nc.dram_tensor("wq", [D, FPC], BF16, kind="ExternalInput").ap()
    wk = nc.dram_tensor("wk", [D, FPC], BF16, kind="ExternalInput").ap()
    wv = nc.dram_tensor("wv", [D, FPC], BF16, kind="ExternalInput").ap()
    wo = nc.dram_tensor("wo", [FPC, D], BF16, kind="ExternalInput").ap()
    mask = nc.dram_tensor("mask", [2, 128], BF16, kind="ExternalInput").ap()
    out = nc.dram_tensor("out", [S, D], BF16, kind="ExternalOutput").ap()

    with tile.TileContext(nc) as tc:
        with (
            tc.tile_pool(name="wp", bufs=1) as wp,
            tc.tile_pool(name="qkv", bufs=1) as qkv,
            tc.tile_pool(name="ptp", bufs=5) as ptp,
            tc.tile_pool(name="otp", bufs=2) as otp,
            tc.tile_pool(name="smalls", bufs=2) as smalls,
            tc.tile_pool(name="outp", bufs=3) as outp,
            tc.tile_pool(name="xqp", bufs=2) as xqp,
            tc.tile_pool(name="xp", bufs=1) as xp,
            tc.tile_pool(name="sp", bufs=2, space="PSUM") as sp,
            tc.tile_pool(name="avp", bufs=2, space="PSUM") as avp,
            tc.tile_pool(name="miscp", bufs=2, space="PSUM") as miscp,
        ):
            wq_sb = wp.tile([128, NB, NP, 128], BF16, tag="wq")
            wk_sb = wp.tile([128, NB, NP, 128], BF16, tag="wk")
            wv_sb = wp.tile([128, NB, FPC], BF16, tag="wv")
            wo_sb = wp.tile([128, NP, D], BF16, tag="wo")
            m_sb = wp.tile([2, 128], BF16, tag="mask")
            warm = wp.tile([64, 64], BF16, tag="warm")

            qt_sb = qkv.tile([128, NP, S], BF16, tag="qt")
            kt_sb = qkv.tile([128, NP, S], BF16, tag="kt")
            v_sb = qkv.tile([128, NKB, HPC, 65], BF16, tag="v")
            nc.vector.memset(warm[:], 0.0)
            nc.vector.memset(v_sb[:, :, :, 64], 1.0)

            xk_sb = xp.tile([128, NB, S], BF16, tag="xk", name="xk_sb")
            xv_sb = xp.tile([128, NB, S], BF16, tag="xv", name="xv_sb")

            # xq lives as 512-token quarters (one per q chunk); quarters 2/3
            # are allocated+DMA'd mid-kernel reusing quarters 0/1's slots.
            xqq = {}

            def alloc_xq_quarter(c):
                t = xqp.tile([128, NB, QC], BF16, tag="xqh", name="xqh")
                xqq[c] = t
                for b in range(NB):
                    nc.sync.dma_start(
                        t[:, b], xq[b * 128:(b + 1) * 128,
                                    c * QC:(c + 1) * QC])

            # ---- DMA emission: priority order for the lead-in ----
            # m0-first weight slices and token-quarter x slices so the first
            # scores/exp only wait on ~2.5MB instead of the full 16MB.
            def bsl_(b):
                return slice(b * 128, (b + 1) * 128)

            nc.sync.dma_start(m_sb[:], mask)
            for b in range(NB):
                nc.sync.dma_start(
                    wq_sb[:, b],
                    wq[bsl_(b), :].rearrange("p (m c) -> p m c", c=128))
            alloc_xq_quarter(0)
            for b in range(NB):
                nc.sync.dma_start(
                    wk_sb[:, b],
                    wk[bsl_(b), :].rearrange("p (m c) -> p m c", c=128))
            for b in range(NB):
                nc.sync.dma_start(xk_sb[:, b, 0:QC], xk[bsl_(b), 0:QC])
            for b in range(NB):
                nc.sync.dma_start(xk_sb[:, b, QC:S], xk[bsl_(b), QC:S])
            for b in range(NB):
                nc.sync.dma_start(wv_sb[:, b], wv[bsl_(b), :])
            for b in range(NB):
                nc.sync.dma_start(xv_sb[:, b, 0:QC], xv[bsl_(b), 0:QC])
            for c in range(1, 4):
                for b in range(NB):
                    nc.sync.dma_start(xv_sb[:, b, c * QC:(c + 1) * QC],
                                      xv[bsl_(b), c * QC:(c + 1) * QC])
            alloc_xq_quarter(1)
            for fb in range(NP):
                nc.sync.dma_start(wo_sb[:, fb], wo[fb * 128:(fb + 1) * 128, :])

            # ---- filler thunk builders ----
            def q_chunk_thunk(m, c):
                def th(m=m, c=c):
                    ps = miscp.tile([128, QC], F32, tag="misc", name="pq")
                    for b in range(NB):
                        nc.tensor.matmul(
                            ps[:], wq_sb[:, b, m], xqq[c][:, b, :],
                            start=(b == 0), stop=(b == NB - 1))
                    nc.vector.tensor_copy(
                        qt_sb[:, m, c * QC:(c + 1) * QC], ps[:])
                return th

            def k_chunk_mms(m, c):
                st = {}
                ths = []
                for b in range(NB):
                    def mm(b=b, m=m, c=c):
                        if b == 0:
                            st["ps"] = miscp.tile([128, QC], F32, tag="misc",
                                                  name="pk")
                        nc.tensor.matmul(
                            st["ps"][:], wk_sb[:, b, m],
                            xk_sb[:, b, c * QC:(c + 1) * QC],
                            start=(b == 0), stop=(b == NB - 1))
                        if b == NB - 1:
                            nc.vector.tensor_copy(
                                kt_sb[:, m, c * QC:(c + 1) * QC], st["ps"][:])
                    ths.append(mm)
                return ths

            def v_tt_thunk(tt):
                def th(tt=tt):
                    ps = miscp.tile([128, FPC], F32, tag="misc", name="pv")
                    for b in range(NB):
                        nc.tensor.matmul(
                            ps[:], xv_sb[:, b, tt * 128:(tt + 1) * 128],
                            wv_sb[:, b],
                            start=(b == 0), stop=(b == NB - 1))
                    nc.vector.tensor_copy(
                        v_sb[:, tt, :, 0:64],
                        ps[:].rearrange("p (h c) -> p h c", c=64))
                return th

            ot_tiles = {}

            def wo_thunks(qc_w, tt):
                ot_w = ot_tiles[qc_w]
                st = {}
                ths = []
                for jc in range(2):
                    for fb in range(NP):
                        def mm(jc=jc, fb=fb, qc_w=qc_w, tt=tt):
                            if fb == 0:
                                st[jc] = miscp.tile([128, QC], F32, tag="misc",
                                                    name="wop")
                                if jc == 0:
                                    st["o"] = outp.tile([128, D], BF16,
                                                        tag="ostage",
                                                        name="ostage")
                            tsl = slice(tt * 128, (tt + 1) * 128)
                            nc.tensor.matmul(
                                st[jc][:], ot_w[fb][:, tsl],
                                wo_sb[:, fb, jc * 512:(jc + 1) * 512],
                                start=(fb == 0), stop=(fb == NP - 1))
                            if fb == NP - 1:
                                nc.vector.tensor_copy(
                                    st["o"][:, jc * 512:(jc + 1) * 512],
                                    st[jc][:])
                                if jc == 1:
                                    row = qc_w * QC + tt * 128
                                    nc.sync.dma_start(out[row:row + 128, :],
                                                      st["o"][:])
                        ths.append(mm)
                return ths

            def finish_pair(job):
                ot_t, av_sb_t, rec2_t = job
                scp = miscp.tile([128, QC], F32, tag="misc", name="scp")
                nc.tensor.matmul(scp[:], m_sb[:], rec2_t[:], start=True,
                                 stop=True)
                nc.vector.tensor_mul(ot_t[0:64, :], av_sb_t[0:64, 0:QC],
                                     scp[0:64, :])
                nc.vector.tensor_mul(ot_t[64:128, :],
                                     av_sb_t[0:64, QC:2 * QC], scp[64:128, :])

            def put(fill, start_slot, thunks, per_slot=2):
                for i, th in enumerate(thunks):
                    fill[min(start_slot + i // per_slot, NKB - 1)].append(th)

            def put_stride(fill, thunks, first, stride):
                for i, th in enumerate(thunks):
                    fill[min(first + stride * i, NKB - 1)].append(th)

            # ---- lead-in ----
            # ~160 dummy matmuls (~9us) keep the PE HAM-warm while the input
            # DMAs land, so the real projections start at 2.4GHz.
            NWARM = 110
            wps = miscp.tile([64, 64], F32, tag="misc", name="warmps")
            for i in range(NWARM):
                nc.tensor.matmul(wps[0:64, 0:64], warm[:], warm[:],
                                 start=(i == 0), stop=(i == NWARM - 1))
            q_chunk_thunk(0, 0)()
            for c in range(4):
                for th in k_chunk_mms(0, c):
                    th()
            for c in range(4):
                for th in k_chunk_mms(1, c):
                    th()
            for tt in range(4):
                v_tt_thunk(tt)()

            # ---- attention (qc, m) with interleaved fillers ----
            # The last LAG attnV matmuls + av evacuation of window W run in
            # the first slots of window W+1 (ring), so the PE never drains
            # waiting for W's last exp before starting W+1's scores.
            pending = None
            carry = []
            for qc in range(NQC):
                ot_tiles[qc] = {}
                qsl = slice(qc * QC, (qc + 1) * QC)
                for m in range(NP):
                    ot = otp.tile([128, QC], BF16, tag="ot%d" % m, name="ot")
                    ot_tiles[qc][m] = ot
                    fill = [[] for _ in range(NKB)]
                    if qc == 0:
                        if m == 0:
                            for i, tt in enumerate(range(4, 16)):
                                fill[i].append(v_tt_thunk(tt))
                            fill[12].append(q_chunk_thunk(1, 0))
                        else:
                            if m < NP - 1:
                                for c in range(4):
                                    put(fill, c * 3, k_chunk_mms(m + 1, c),
                                        per_slot=3)
                                fill[12].append(q_chunk_thunk(m + 1, 0))
                            else:
                                fill[12].append(q_chunk_thunk(0, 1))
                    else:
                        if m == 0 and qc >= 1 and qc + 1 < NQC:
                            fill[0].append(
                                lambda c=qc + 1: alloc_xq_quarter(c))
                        put_stride(fill, wo_thunks(qc - 1, m), 2, 2)
                        if m < NP - 1:
                            fill[9].append(q_chunk_thunk(m + 1, qc))
                        elif qc < NQC - 1:
                            fill[9].append(q_chunk_thunk(0, qc + 1))

                    avA = avp.tile([128, QC], F32, tag="av", name="avA")
                    avB = avp.tile([128, QC], F32, tag="av", name="avB")
                    pts = {}

                    def do_attnv(j, m=m, avA=avA, avB=avB, pts=pts):
                        nc.tensor.matmul(
                            avA[0:65, :], v_sb[:, j, 2 * m, 0:65],
                            pts[j][:, 0:512],
                            start=(j == 0), stop=(j == NKB - 1))
                        nc.tensor.matmul(
                            avB[0:65, :], v_sb[:, j, 2 * m + 1, 0:65],
                            pts[j][:, 512:1024],
                            start=(j == 0), stop=(j == NKB - 1))

                    def evac_pair(ot=ot, avA=avA, avB=avB):
                        nonlocal pending
                        av_sb = smalls.tile([128, 1024], F32, tag="av_sb",
                                            name="av_sb")
                        nc.vector.tensor_copy(av_sb[0:65, 0:QC], avA[0:65, :])
                        nc.vector.tensor_copy(av_sb[0:65, QC:2 * QC],
                                              avB[0:65, :])
                        den2 = smalls.tile([2, QC], F32, tag="den2",
                                           name="den2")
                        nc.sync.dma_start(den2[0:2, :], av_sb[64:65, 0:2 * QC])
                        recf = smalls.tile([2, QC], F32, tag="recf",
                                           name="recf")
                        nc.vector.reciprocal_approx_fast(recf[:], den2[:])
                        rec2 = smalls.tile([2, QC], BF16, tag="rec2",
                                           name="rec2")
                        nc.vector.tensor_copy(rec2[:], recf[:])
                        pending = (ot, av_sb, rec2)

                    for kb in range(NKB):
                        s = sp.tile([128, 1024], F32, tag="s", name="s")
                        ksl = slice(kb * 128, (kb + 1) * 128)
                        nc.tensor.matmul(s[:, 0:512], kt_sb[0:64, m, ksl],
                                         qt_sb[0:64, m, qsl],
                                         start=True, stop=True,
                                         tile_position=(0, 0))
                        nc.tensor.matmul(s[:, 512:1024], kt_sb[64:128, m, ksl],
                                         qt_sb[64:128, m, qsl],
                                         start=True, stop=True,
                                         tile_position=(64, 0))
                        pt = ptp.tile([128, 1024], BF16, tag="pt", name="pt")
                        nc.scalar.activation(pt[:], s[:], EXP, scale=0.125)
                        pts[kb] = pt
                        # order: scores -> fillers -> attnV. The scores
                        # row-pair streams 216ns but needs 2x107ns of weight
                        # loads; a filler matmul in between gives the load
                        # path stream-time to hide the attnV weight loads.
                        if kb < len(carry):
                            carry[kb]()
                        if kb == 8 and pending is not None:
                            finish_pair(pending)
                            pending = None
                        for f in fill[kb]:
                            f()
                        if kb >= LAG:
                            do_attnv(kb - LAG)

                    def mk_drain(j, last, do_attnv=do_attnv,
                                 evac_pair=evac_pair):
                        def d():
                            do_attnv(j)
                            if last:
                                evac_pair()
                        return d

                    carry = [mk_drain(j, j == NKB - 1)
                             for j in range(NKB - LAG, NKB)]

            # drain: last pair, its normalization, last q chunk's Wo.
            # tt0's fb0-2 matmuls only read already-normalized pairs, so they
            # run while the last pair's reciprocal chain completes.
            for th in carry:
                th()
            wps2 = miscp.tile([64, 64], F32, tag="misc", name="warmps2")
            for i in range(20):
                nc.tensor.matmul(wps2[0:64, 0:64], warm[:], warm[:],
                                 start=(i == 0), stop=(i == 19))
            tail0 = wo_thunks(NQC - 1, 0)
            for i in (0, 1, 2):
                tail0[i]()
            finish_pair(pending)
            for i in (3, 4, 5, 6, 7):
                tail0[i]()
            for tt in range(1, 4):
                for th in wo_thunks(NQC - 1, tt):
                    th()

    nc.compile()
    return nc


def _get_nc():
    global _nc_cache
    if _nc_cache is None:
        _nc_cache = build()
    return _nc_cache


def kernel(query, key, value, W_q, W_k, W_v, W_o):
    global last_results
    nc = _get_nc()
    bf = ml_dtypes.bfloat16

    mask = np.zeros((2, 128), bf)
    mask[0, 0:64] = 1.0
    mask[1, 64:128] = 1.0

    in_maps = []
    xt = {}
    for b in range(B):
        xt[b] = {
            "xq": np.ascontiguousarray(query[b].T).astype(bf),
            "xk": np.ascontiguousarray(key[b].T).astype(bf),
            "xv": np.ascontiguousarray(value[b].T).astype(bf),
        }
    wmaps = []
    for hg in range(2):
        r = slice(hg * FPC, (hg + 1) * FPC)
        wmaps.append({
            "wq": np.ascontiguousarray(W_q[r, :].T).astype(bf),
            "wk": np.ascontiguousarray(W_k[r, :].T).astype(bf),
            "wv": np.ascontiguousarray(W_v[r, :].T).astype(bf),
            "wo": np.ascontiguousarray(W_o[:, r].T).astype(bf),
        })
    for c in range(8):
        b, hg = c // 2, c % 2
        in_maps.append({**xt[b], **wmaps[hg], "mask": mask})

    res = run_bass_kernel_spmd(
        nc, in_maps, core_ids=list(range(8)),
        trace=bool(os.environ.get("BASS_KERNEL_TRACE")))
    last_results = res

    out = np.empty((B, S, D), np.float32)
    for b in range(B):
        out[b] = (res.results[2 * b]["out"].astype(np.float32)
                  + res.results[2 * b + 1]["out"].astype(np.float32))
    return out
